# revision 8
# baseline (speedup 1.0000x reference)
"""Trainium2 Bass kernel for nn_BoundaryLoss (boundary EDT + weighted L1 loss).

Strategy (pure data parallel, 1 image per NeuronCore, 8 cores):
  Per image on device:
    nbB   = (t <= 0.5) * BIG                  (scaled background mask)
    dr    = max3_h(nbB)                       (horizontal dilation)
    er_hB = BIG - dr                          (scaled horizontal erosion of fg)
    transpose nbB, er_hB (PE)
    er3B  = min3_v(er_hBT)                    (full 3x3 erosion, scaled)
    fv    = max(nbBT, er3B)                   (0 on boundary, BIG elsewhere)
    g     = windowed (+-3) vertical L1 distance via log-doubling
    g2T   = g*g, transpose back (PE)
    d2    = min_{|u|<=3} g2[j+u] + u^2        (windowed exact EDT, K=3)
    diff2 = (sigmoid(pred) - t)^2             (ACT Square)
    outb[:,0]   = sum sqrt(d2*diff2)          (ACT sqrt with accum_out)
    outb[:,1:3] = max d2 per chunk            (DVE reduce)
  Host: verify windowed-exactness bound (m2 <= 9), normalize, batch mean;
  exact numpy fallback for any image failing the bound (never on dense masks).

Engine split: DVE runs the serial mask/EDT chain (bf16, all-SBUF so
tensor_tensor ops hit the 2x mode and tensor_scalar the 4x mode;
scalar_tensor_tensor is avoided -- it runs 1x). ACT handles sigmoid,
4 PSUM copies, Square, and the final fused sqrt+accumulate (one act-table
switch, overlapped with DVE work). PE does the 12 transposes. GpSimd is
kept idle (measured ~6x slower than DVE and it contends for SBUF ports).

All dtypes bf16 except the target (f32, exact 0.5 compare) and the f32
accumulators. Identity ships as a bf16 DRAM tensor (no cast copy).
"""

import os
from contextlib import ExitStack

import numpy as np

H = 256
W = 256
P = 128
C = 2  # partition chunks per image (H = C * P)
KH = 3  # phase-2 horizontal window (exactness proof bound: m2 <= KH*KH)
BIGF = 16384.0  # phase-1 sentinel (bf16-exact; BIGF + small stays BIGF in bf16)
BIG2 = 1.0e9  # phase-2 border pad, > BIGF^2
PAD1 = 4  # phase-1 doubling pads (window 1+2)
FW = H + 2 * PAD1
GW = W + 2 * KH

LAST_RESULTS = None  # BassKernelResults of the most recent device run


def _build_nc():
    import concourse.bass as bass
    import concourse.mybir as mybir

    bf16 = mybir.dt.bfloat16
    f32 = mybir.dt.float32
    Alu = mybir.AluOpType
    Act = mybir.ActivationFunctionType

    nc = bass.Bass(detect_race_conditions=False)
    tgt_d = nc.dram_tensor("tgt", [P, C * W], bf16, kind="ExternalInput")
    prd_d = nc.dram_tensor("prd", [P, C * W], bf16, kind="ExternalInput")
    idn_d = nc.dram_tensor("idn", [P, P], bf16, kind="ExternalInput")
    out_d = nc.dram_tensor("out", [P, 4], f32, kind="ExternalOutput")

    ctx = ExitStack()
    sb = lambda name, shape, dt: ctx.enter_context(nc.sbuf_tensor(name, shape, dt))

    with ctx:
        tgt = sb("tgt_t", [P, C, W], bf16)
        prd = sb("prd_t", [P, C, W], bf16)
        ident = sb("ident", [P, P], bf16)
        nbp = sb("nbp", [P, C, W + 2], bf16)  # padded nbB (pad cols = 0)
        t1 = sb("t1", [P, C, W], bf16)
        dr = sb("dr", [P, C, W], bf16)
        erh = sb("erh", [P, C, W], bf16)
        nbT = sb("nbT", [P, C, H], bf16)
        erTp = sb("erTp", [P, C, H + 2], bf16)  # padded er_hBT (pads = BIGF)
        t2 = sb("t2", [P, C, H], bf16)
        er3 = sb("er3", [P, C, H], bf16)
        fvA = sb("fvA", [P, C, FW], bf16)
        fvB = sb("fvB", [P, C, FW], bf16)
        tmpd = sb("tmpd", [P, C, FW], bf16)
        tmp2 = sb("tmp2", [P, C, FW], bf16)
        g2T = sb("g2T", [P, C, H], bf16)
        g2p = sb("g2p", [P, C, GW], bf16)  # padded g2 (pads = BIG2)
        p2m = sb("p2m", [P, C, W], bf16)
        p2h = sb("p2h", [P, C, W], bf16)
        acc1 = sb("acc1", [P, C, W], bf16)
        acc2 = sb("acc2", [P, C, W], bf16)
        d2 = sb("d2", [P, C, W], bf16)
        sg = sb("sg", [P, C, W], bf16)
        diff = sb("diff", [P, C, W], bf16)
        diff2 = sb("diff2", [P, C, W], bf16)
        ddf = sb("ddf", [P, C, W], bf16)
        junk = sb("junk", [P, C, W], bf16)
        outb = sb("outb", [P, 4], f32)
        warm = sb("warm", [P, 4], f32)
        blks = [ctx.enter_context(nc.psum_tensor(f"blk{i}", [P, P], bf16)) for i in range(8)]

        dma_t = ctx.enter_context(nc.semaphore("dma_t"))
        dma_p = ctx.enter_context(nc.semaphore("dma_p"))
        dma_i = ctx.enter_context(nc.semaphore("dma_i"))
        dma_o = ctx.enter_context(nc.semaphore("dma_o"))
        dve_s = ctx.enter_context(nc.semaphore("dve_s"))
        act_s = ctx.enter_context(nc.semaphore("act_s"))
        pe_s = ctx.enter_context(nc.semaphore("pe_s"))
        w_s = ctx.enter_context(nc.semaphore("w_s"))

        # --- main-body (pre-Block) instructions: execute ~1us before the
        # block-entry barrier, so input DMAs + act-table warm start early ---
        nc.sync.dma_start(out=tgt[:], in_=tgt_d[:]).then_inc(dma_t, 16)
        nc.scalar.dma_start(out=prd[:], in_=prd_d[:]).then_inc(dma_p, 16)
        nc.scalar.dma_start(out=ident[:], in_=idn_d[:]).then_inc(dma_i, 16)
        nc.vector.memset(warm[:, 0:1], 1.0).then_inc(w_s, 1)
        nc.vector.memset(nbp[:, :, 0:1], 0.0)
        nc.vector.memset(nbp[:, :, W + 1 : W + 2], 0.0)
        nc.vector.memset(erTp[:, :, 0:1], BIGF)
        nc.vector.memset(erTp[:, :, H + 1 : H + 2], BIGF)
        nc.vector.memset(fvA[:, :, 0:PAD1], BIGF)
        nc.vector.memset(fvA[:, :, PAD1 + H : FW], BIGF)
        nc.vector.memset(fvB[:, :, 0:PAD1], BIGF)
        nc.vector.memset(fvB[:, :, PAD1 + H : FW], BIGF)
        nc.vector.memset(g2p[:, :, 0:KH], BIG2)
        nc.vector.memset(g2p[:, :, KH + W : GW], BIG2)
        nc.vector.memset(outb[:, 3:4], 0.0)
        # ACT: warm the sigmoid-set table while the DMAs run
        nc.scalar.wait_ge(w_s, 1)
        nc.scalar.activation(warm[:, 1:2], warm[:, 0:1], Act.Sigmoid)
        block = ctx.enter_context(nc.Block(no_gpsimd_drain=True))

        # dve_s: 1=nbB 2=er_hB 3=diff 4-7=g2T quarter blocks 8=ddf 9=m2
        # act_s: 1=sg 2=nbT copied 3=diff2 4=final accum
        # pe_s: 1-4 nbB T, 5-8 er T, 9-12 g2 T

        @block.sync
        def _(sync: "bass.BassEngine"):
            sync.wait_ge(dve_s, 9)
            sync.wait_ge(act_s, 4)
            sync.dma_start(out=out_d[:], in_=outb[:]).then_inc(dma_o, 16)
            sync.wait_ge(dma_o, 16)

        @block.scalar
        def _(scalar: "bass.BassEngine"):
            scalar.wait_ge(dma_p, 16)
            nc.scalar.activation(sg[:], prd[:], Act.Sigmoid).then_inc(act_s, 1)  # a=1
            # nbT copies from PSUM banks 0-3
            k = 0
            for wb in range(C):
                for hc in range(C):
                    scalar.wait_ge(pe_s, k + 1)
                    ins = nc.scalar.copy(nbT[:, wb, hc * P : (hc + 1) * P], blks[k][:])
                    k += 1
            ins.then_inc(act_s, 1)  # a=2
            # diff2 = Square(diff) (same act table set)
            scalar.wait_ge(dve_s, 3)
            nc.scalar.activation(diff2[:], diff[:], Act.Square).then_inc(act_s, 1)  # a=3
            # switch table to the sqrt set now (overlaps DVE doubling/phase 2)
            nc.scalar.sqrt(warm[:, 1:2], warm[:, 0:1])
            # final: outb[:,0] = sum sqrt(d2*diff^2) = sum dist*|diff|
            scalar.wait_ge(dve_s, 8)
            nc.scalar.activation(
                junk[:], ddf[:], Act.Sqrt, accum_out=outb[:, 0:1]
            ).then_inc(act_s, 1)  # a=4

        @block.tensor
        def _(tensor: "bass.BassEngine"):
            # nbB transposes (read padded buffer interior)
            tensor.wait_ge(dma_i, 16)
            tensor.wait_ge(dve_s, 1)
            k = 0
            for wb in range(C):
                for hc in range(C):
                    nc.tensor.transpose(
                        blks[k][:], nbp[:, hc, 1 + wb * P : 1 + (wb + 1) * P], ident[:]
                    ).then_inc(pe_s, 1)
                    k += 1
            # er_hB transposes
            tensor.wait_ge(dve_s, 2)
            for wb in range(C):
                for hc in range(C):
                    nc.tensor.transpose(
                        blks[k][:], erh[:, hc, wb * P : (wb + 1) * P], ident[:]
                    ).then_inc(pe_s, 1)
                    k += 1
            # g2 transpose-back: banks 0-3 free once ACT copied nbT (a>=2)
            tensor.wait_ge(act_s, 2)
            for k in range(4):
                wb, hc = divmod(k, C)
                tensor.wait_ge(dve_s, 4 + k)
                nc.tensor.transpose(
                    blks[k][:], g2T[:, wb, hc * P : (hc + 1) * P], ident[:]
                ).then_inc(pe_s, 1)

        @block.vector
        def _(vector: "bass.BassEngine"):
            vector.wait_ge(dma_t, 16)
            # nbB = (t <= 0.5) * BIG, written into the padded buffer
            nc.vector.tensor_scalar(
                nbp[:, :, 1 : W + 1], tgt[:], 0.5, BIGF, Alu.is_le, Alu.mult
            ).then_inc(dve_s, 1)  # d=1
            # horizontal dilation (max3) of scaled background
            nc.vector.tensor_tensor(t1[:], nbp[:, :, 0:W], nbp[:, :, 2 : W + 2], Alu.max)
            nc.vector.tensor_tensor(dr[:], t1[:], nbp[:, :, 1 : W + 1], Alu.max)
            # er_hB = BIG - dr (scaled horizontal erosion of foreground)
            nc.vector.tensor_scalar(
                erh[:], dr[:], -1.0, BIGF, Alu.mult, Alu.add
            ).then_inc(dve_s, 1)  # d=2
            # diff = sigmoid(pred) - t (fills the PE-transpose gap)
            vector.wait_ge(act_s, 1)
            nc.vector.tensor_tensor(diff[:], sg[:], tgt[:], Alu.subtract).then_inc(dve_s, 1)  # d=3
            # er_hBT copies from PSUM banks 4-7
            for k in range(4):
                vector.wait_ge(pe_s, 5 + k)
                wb, hc = divmod(k, C)
                nc.vector.tensor_copy(erTp[:, wb, 1 + hc * P : 1 + (hc + 1) * P], blks[4 + k][:])
            # er3B = min3_v(er_hBT); fv = max(nbT, er3B): 0 on boundary, BIG else
            nc.vector.tensor_tensor(t2[:], erTp[:, :, 0:H], erTp[:, :, 2 : H + 2], Alu.min)
            nc.vector.tensor_tensor(er3[:], t2[:], erTp[:, :, 1 : H + 1], Alu.min)
            vector.wait_ge(act_s, 2)
            nc.vector.tensor_tensor(fvA[:, :, PAD1 : PAD1 + H], er3[:], nbT[:], Alu.max)
            # vertical L1 distance by log-doubling (window 1+2 = 3)
            cur, nxt = fvA, fvB
            for d in (1, 2):
                lo, hi = d, FW - d
                nc.vector.tensor_tensor(
                    tmpd[:, :, lo:hi], cur[:, :, 0 : FW - 2 * d], cur[:, :, 2 * d : FW], Alu.min
                )
                nc.vector.tensor_scalar(
                    tmp2[:, :, lo:hi], tmpd[:, :, lo:hi], float(d), None, Alu.add
                )
                nc.vector.tensor_tensor(
                    nxt[:, :, lo:hi], tmp2[:, :, lo:hi], cur[:, :, lo:hi], Alu.min
                )
                cur, nxt = nxt, cur
            # square the vertical distance, one quarter block at a time so
            # the PE transpose-back pipelines behind each block
            for wb in range(C):
                for hc in range(C):
                    nc.vector.tensor_tensor(
                        g2T[:, wb, hc * P : (hc + 1) * P],
                        cur[:, wb, PAD1 + hc * P : PAD1 + (hc + 1) * P],
                        cur[:, wb, PAD1 + hc * P : PAD1 + (hc + 1) * P],
                        Alu.mult,
                    ).then_inc(dve_s, 1)  # d=4..7
            # g2p copies (all on DVE; ACT copies measured 2x slower)
            for k in range(4):
                wb, hc = divmod(k, C)
                vector.wait_ge(pe_s, 9 + k)
                nc.vector.tensor_copy(g2p[:, hc, KH + wb * P : KH + (wb + 1) * P], blks[k][:])
            # phase 2: d2 = min_{|u|<=KH} g2[j+u] + u^2
            prev = None
            accs = [acc1, acc2, d2]
            for u in range(1, KH + 1):
                nc.vector.tensor_tensor(
                    p2m[:],
                    g2p[:, :, KH - u : KH - u + W],
                    g2p[:, :, KH + u : KH + u + W],
                    Alu.min,
                )
                nc.vector.tensor_scalar(p2h[:], p2m[:], float(u * u), None, Alu.add)
                base = g2p[:, :, KH : KH + W] if prev is None else prev[:]
                nc.vector.tensor_tensor(accs[u - 1][:], p2h[:], base, Alu.min)
                prev = accs[u - 1]
            # ddf = d2 * diff^2 (unblocks the ACT sqrt-accum tail)
            vector.wait_ge(act_s, 3)
            nc.vector.tensor_tensor(ddf[:], d2[:], diff2[:], Alu.mult).then_inc(dve_s, 1)  # d=8
            nc.vector.tensor_reduce(
                out=outb[:, 1:3], in_=d2[:], axis=mybir.AxisListType.X, op=Alu.max
            ).then_inc(dve_s, 1)  # d=9

    return nc


_NC_CACHE = {}


def _get_nc():
    if "nc" not in _NC_CACHE:
        _NC_CACHE["nc"] = _build_nc()
    return _NC_CACHE["nc"]


# ---------- exact numpy fallback (pathological images only) ----------

def _reference_image_np(t, p):
    """Exact replica of the jax reference for one image, in numpy fp32."""
    b = (t > 0.5).astype(np.float32)
    if not (b > 0).any():
        return 0.0
    v = b.copy()
    v[1:] = np.minimum(v[1:], b[:-1])
    v[:-1] = np.minimum(v[:-1], b[1:])
    er = v.copy()
    er[:, 1:] = np.minimum(er[:, 1:], v[:, :-1])
    er[:, :-1] = np.minimum(er[:, :-1], v[:, 1:])
    bound = b - er
    if bound.sum() == 0:
        bound = b
    feat = bound > 0.5
    BIGV = np.float32(1e6)
    c = np.full(W, BIGV, np.float32)
    d_fwd = np.empty((H, W), np.float32)
    for i in range(H):
        c = np.where(feat[i], np.float32(0.0), c + 1)
        d_fwd[i] = c
    c = np.full(W, BIGV, np.float32)
    d_bwd = np.empty((H, W), np.float32)
    for i in range(H - 1, -1, -1):
        c = np.where(feat[i], np.float32(0.0), c + 1)
        d_bwd[i] = c
    g = np.minimum(d_fwd, d_bwd)
    j = np.arange(W, dtype=np.float32)
    d2 = np.empty((H, W), np.float32)
    for i in range(H):
        d2[i] = np.min(g[i][None, :] ** 2 + (j[:, None] - j[None, :]) ** 2, axis=-1)
    dist = np.sqrt(d2)
    m = dist.max()
    if m > 0:
        dist = dist / (m + np.float32(1e-8))
    sgm = 1.0 / (1.0 + np.exp(-p.astype(np.float64)))
    return float(np.mean(dist * np.abs(sgm - t)))


def _bound_empty(t):
    """True if erosion removes every boundary pixel (reference falls back)."""
    b = (t > 0.5).astype(np.float32)
    v = b.copy()
    v[1:] = np.minimum(v[1:], b[:-1])
    v[:-1] = np.minimum(v[:-1], b[1:])
    er = v.copy()
    er[:, 1:] = np.minimum(er[:, 1:], v[:, :-1])
    er[:, :-1] = np.minimum(er[:, :-1], v[:, 1:])
    return (b - er).sum() == 0


# ---------- public entry point ----------

def kernel(pred_logits: np.ndarray, target: np.ndarray) -> np.ndarray:
    global LAST_RESULTS
    import ml_dtypes
    from concourse.bass_utils import run_bass_kernel_spmd

    pred = np.ascontiguousarray(np.asarray(pred_logits, np.float32)[:, 0])
    tgt = np.ascontiguousarray(np.asarray(target, np.float32)[:, 0])
    B = pred.shape[0]
    assert pred.shape == (B, H, W) and tgt.shape == (B, H, W)
    assert B == 8, f"kernel is built for batch 8, got {B}"

    ident = np.eye(P, dtype=ml_dtypes.bfloat16)

    nc = _get_nc()
    in_maps = []
    for i in range(B):
        tp = np.concatenate([tgt[i, :P], tgt[i, P:]], axis=1).astype(ml_dtypes.bfloat16)
        pp = np.concatenate([pred[i, :P], pred[i, P:]], axis=1).astype(ml_dtypes.bfloat16)
        in_maps.append({"tgt": tp, "prd": pp, "idn": ident})
    trace = bool(int(os.environ.get("KERNEL_TRACE", "0")))
    res = run_bass_kernel_spmd(nc, in_maps, core_ids=list(range(B)), trace=trace)
    LAST_RESULTS = res

    total = 0.0
    for i in range(B):
        o = np.asarray(res.results[i]["out"], np.float32)  # [128, 4]
        if not (tgt[i] > 0.5).any():
            continue  # empty mask: reference skips (loss 0)
        m2 = float(o[:, 1:3].max())
        if m2 > float(KH * KH) or _bound_empty(tgt[i]):
            # windowed EDT not provably exact for this image -> exact path
            total += _reference_image_np(tgt[i], pred[i])
            continue
        S = float(o[:, 0].sum(dtype=np.float64))
        m = np.float32(np.sqrt(np.float32(m2)))
        denom = float(m + np.float32(1e-8)) if m > 0 else 1.0
        total += (S / denom) / float(H * W)
    return np.float32(total / max(B, 1))


# revision 10
# speedup vs baseline: 1.0504x; 1.0504x over previous
"""Trainium2 Bass kernel for nn_BoundaryLoss (boundary EDT + weighted L1 loss).

Strategy (pure data parallel, 1 image per NeuronCore, 8 cores):
  Per image on device:
    nbB   = (t <= 0.5) * BIG                  (scaled background mask)
    dr    = max3_h(nbB)                       (horizontal dilation)
    er_hB = BIG - dr                          (scaled horizontal erosion of fg)
    transpose nbB, er_hB (PE)
    er3B  = min3_v(er_hBT)                    (full 3x3 erosion, scaled)
    fv    = max(nbBT, er3B)                   (0 on boundary, BIG elsewhere)
    g     = windowed (+-3) vertical L1 distance via log-doubling
    g2T   = g*g, transpose back (PE)
    d2    = min_{|u|<=3} g2[j+u] + u^2        (windowed exact EDT, K=3)
    diff2 = (sigmoid(pred) - t)^2             (ACT Square)
    outb[:,0]   = sum sqrt(d2*diff2)          (ACT sqrt with accum_out)
    outb[:,1:3] = max d2 per chunk            (DVE reduce)
  Host: verify windowed-exactness bound (m2 <= 9), normalize, batch mean;
  exact numpy fallback for any image failing the bound (never on dense masks).

Engine split: DVE runs the serial mask/EDT chain (bf16, all-SBUF so
tensor_tensor ops hit the 2x mode and tensor_scalar the 4x mode;
scalar_tensor_tensor is avoided -- it runs 1x). ACT handles sigmoid,
4 PSUM copies, Square, and the final fused sqrt+accumulate (one act-table
switch, overlapped with DVE work). PE does the 12 transposes. GpSimd is
kept idle (measured ~6x slower than DVE and it contends for SBUF ports).

All dtypes bf16 except the target (f32, exact 0.5 compare) and the f32
accumulators. Identity ships as a bf16 DRAM tensor (no cast copy).
"""

import os
from contextlib import ExitStack

import numpy as np

H = 256
W = 256
P = 128
C = 2  # partition chunks per image (H = C * P)
KH = 3  # phase-2 horizontal window (exactness proof bound: m2 <= KH*KH)
BIGF = 16384.0  # phase-1 sentinel (bf16-exact; BIGF + small stays BIGF in bf16)
BIG2 = 1.0e9  # phase-2 border pad, > BIGF^2
PAD1 = 4  # phase-1 doubling pads (window 1+2)
FW = H + 2 * PAD1
GW = W + 2 * KH

LAST_RESULTS = None  # BassKernelResults of the most recent device run


def _build_nc():
    import concourse.bass as bass
    import concourse.mybir as mybir

    bf16 = mybir.dt.bfloat16
    f32 = mybir.dt.float32
    Alu = mybir.AluOpType
    Act = mybir.ActivationFunctionType

    nc = bass.Bass(detect_race_conditions=False)
    tgt_d = nc.dram_tensor("tgt", [P, C * W], bf16, kind="ExternalInput")
    prd_d = nc.dram_tensor("prd", [P, C * W], bf16, kind="ExternalInput")
    idn_d = nc.dram_tensor("idn", [P, P], bf16, kind="ExternalInput")
    out_d = nc.dram_tensor("out", [P, 4], f32, kind="ExternalOutput")

    ctx = ExitStack()
    sb = lambda name, shape, dt: ctx.enter_context(nc.sbuf_tensor(name, shape, dt))

    with ctx:
        tgt = sb("tgt_t", [P, C, W], bf16)
        prd = sb("prd_t", [P, C, W], bf16)
        ident = sb("ident", [P, P], bf16)
        nbp = sb("nbp", [P, C, W + 2], bf16)  # padded nbB (pad cols = 0)
        t1 = sb("t1", [P, C, W], bf16)
        dr = sb("dr", [P, C, W], bf16)
        erh = sb("erh", [P, C, W], bf16)
        nbT = sb("nbT", [P, C, H], bf16)
        erTp = sb("erTp", [P, C, H + 2], bf16)  # padded er_hBT (pads = BIGF)
        t2 = sb("t2", [P, C, H], bf16)
        er3 = sb("er3", [P, C, H], bf16)
        fvA = sb("fvA", [P, C, FW], bf16)
        fvB = sb("fvB", [P, C, FW], bf16)
        tmpd = sb("tmpd", [P, C, FW], bf16)
        tmp2 = sb("tmp2", [P, C, FW], bf16)
        g2T = sb("g2T", [P, C, H], bf16)
        g2p = sb("g2p", [P, C, GW], bf16)  # padded g2 (pads = BIG2)
        p2m = sb("p2m", [P, C, W], bf16)
        p2h = sb("p2h", [P, C, W], bf16)
        acc1 = sb("acc1", [P, C, W], bf16)
        acc2 = sb("acc2", [P, C, W], bf16)
        d2 = sb("d2", [P, C, W], bf16)
        sg = sb("sg", [P, C, W], bf16)
        diff = sb("diff", [P, C, W], bf16)
        diff2 = sb("diff2", [P, C, W], bf16)
        ddf = sb("ddf", [P, C, W], bf16)
        junk = sb("junk", [P, C, W], bf16)
        outb = sb("outb", [P, 4], f32)
        warm = sb("warm", [P, 4], f32)
        blks = [ctx.enter_context(nc.psum_tensor(f"blk{i}", [P, P], bf16)) for i in range(8)]

        dma_t = ctx.enter_context(nc.semaphore("dma_t"))
        dma_p = ctx.enter_context(nc.semaphore("dma_p"))
        dma_i = ctx.enter_context(nc.semaphore("dma_i"))
        dma_o = ctx.enter_context(nc.semaphore("dma_o"))
        dve_s = ctx.enter_context(nc.semaphore("dve_s"))
        act_s = ctx.enter_context(nc.semaphore("act_s"))
        pe_s = ctx.enter_context(nc.semaphore("pe_s"))
        w_s = ctx.enter_context(nc.semaphore("w_s"))

        block = ctx.enter_context(nc.Block(no_gpsimd_drain=True))

        # dve_s: 1=nbB(c0) 2=erh(c0) 3=nbB(c1) 4=erh(c1) 5=diff
        #        6-9=g2T quarter blocks 10=ddf 11=m2
        # act_s: 1=sg 2=nbT copied 3=diff2 4=final accum
        # pe_s:  1,2=nbB(hc0) 3,4=er(hc0) 5,6=nbB(hc1) 7,8=er(hc1) 9-12=g2 T
        # psum:  blk0,1=nbB(hc0 wb0/1) blk2,3=nbB(hc1) blk4,5=er(hc0) blk6,7=er(hc1)

        @block.sync
        def _(sync: "bass.BassEngine"):
            # target ships as two chunk DMAs so stage A pipelines per chunk
            sync.dma_start(out=tgt[:, 0, :], in_=tgt_d[:, 0:W]).then_inc(dma_t, 16)
            sync.dma_start(out=tgt[:, 1, :], in_=tgt_d[:, W : 2 * W]).then_inc(dma_t, 16)
            sync.wait_ge(dve_s, 11)
            sync.wait_ge(act_s, 4)
            sync.dma_start(out=out_d[:], in_=outb[:]).then_inc(dma_o, 16)
            sync.wait_ge(dma_o, 16)

        @block.scalar
        def _(scalar: "bass.BassEngine"):
            nc.scalar.dma_start(out=prd[:], in_=prd_d[:]).then_inc(dma_p, 16)
            nc.scalar.dma_start(out=ident[:], in_=idn_d[:]).then_inc(dma_i, 16)
            # warm the sigmoid-set act table while DMAs run (square is in
            # every set, so sigmoid+square need no reload in between)
            scalar.wait_ge(w_s, 1)
            nc.scalar.activation(warm[:, 1:2], warm[:, 0:1], Act.Sigmoid)
            scalar.wait_ge(dma_p, 16)
            nc.scalar.activation(sg[:], prd[:], Act.Sigmoid).then_inc(act_s, 1)  # a=1
            # nbT copies from PSUM banks 0-3 (pe order: 1,2 then 5,6)
            for blk_i, pe_n, wb, hc in ((0, 1, 0, 0), (1, 2, 1, 0), (2, 5, 0, 1), (3, 6, 1, 1)):
                scalar.wait_ge(pe_s, pe_n)
                ins = nc.scalar.copy(nbT[:, wb, hc * P : (hc + 1) * P], blks[blk_i][:])
            ins.then_inc(act_s, 1)  # a=2
            # diff2 = Square(diff) (same act table set)
            scalar.wait_ge(dve_s, 5)
            nc.scalar.activation(diff2[:], diff[:], Act.Square).then_inc(act_s, 1)  # a=3
            # switch table to the sqrt set now (overlaps DVE doubling/phase 2)
            nc.scalar.sqrt(warm[:, 1:2], warm[:, 0:1])
            # final: outb[:,0] = sum sqrt(d2*diff^2) = sum dist*|diff|
            scalar.wait_ge(dve_s, 10)
            nc.scalar.activation(
                junk[:], ddf[:], Act.Sqrt, accum_out=outb[:, 0:1]
            ).then_inc(act_s, 1)  # a=4

        @block.tensor
        def _(tensor: "bass.BassEngine"):
            # per-chunk stage-A transposes, pipelined behind the DVE A-chain
            tensor.wait_ge(dma_i, 16)
            for hc in range(C):
                tensor.wait_ge(dve_s, 1 + 2 * hc)  # nbB(c) ready
                for wb in range(C):
                    nc.tensor.transpose(
                        blks[2 * hc + wb][:], nbp[:, hc, 1 + wb * P : 1 + (wb + 1) * P], ident[:]
                    ).then_inc(pe_s, 1)
                tensor.wait_ge(dve_s, 2 + 2 * hc)  # erh(c) ready
                for wb in range(C):
                    nc.tensor.transpose(
                        blks[4 + 2 * hc + wb][:], erh[:, hc, wb * P : (wb + 1) * P], ident[:]
                    ).then_inc(pe_s, 1)
            # g2 transpose-back: banks 0-3 free once ACT copied nbT (a>=2)
            tensor.wait_ge(act_s, 2)
            for k in range(4):
                wb, hc = divmod(k, C)
                tensor.wait_ge(dve_s, 6 + k)
                nc.tensor.transpose(
                    blks[k][:], g2T[:, wb, hc * P : (hc + 1) * P], ident[:]
                ).then_inc(pe_s, 1)

        @block.vector
        def _(vector: "bass.BassEngine"):
            # data-independent pad memsets (run before input arrives)
            nc.vector.memset(warm[:, 0:1], 1.0).then_inc(w_s, 1)
            nc.vector.memset(nbp[:, :, 0:1], 0.0)
            nc.vector.memset(nbp[:, :, W + 1 : W + 2], 0.0)
            nc.vector.memset(erTp[:, :, 0:1], BIGF)
            nc.vector.memset(erTp[:, :, H + 1 : H + 2], BIGF)
            nc.vector.memset(fvA[:, :, 0:PAD1], BIGF)
            nc.vector.memset(fvA[:, :, PAD1 + H : FW], BIGF)
            nc.vector.memset(fvB[:, :, 0:PAD1], BIGF)
            nc.vector.memset(fvB[:, :, PAD1 + H : FW], BIGF)
            nc.vector.memset(g2p[:, :, 0:KH], BIG2)
            nc.vector.memset(g2p[:, :, KH + W : GW], BIG2)
            nc.vector.memset(outb[:, 3:4], 0.0)

            # per-chunk A-chain: nbB = (t <= 0.5)*BIG, dil_h, er_hB = BIG - dr
            for hc in range(C):
                vector.wait_ge(dma_t, 16 * (hc + 1))
                nc.vector.tensor_scalar(
                    nbp[:, hc, 1 : W + 1], tgt[:, hc, :], 0.5, BIGF, Alu.is_le, Alu.mult
                ).then_inc(dve_s, 1)  # d=1 (c0), d=3 (c1)
                nc.vector.tensor_tensor(
                    t1[:, hc, :], nbp[:, hc, 0:W], nbp[:, hc, 2 : W + 2], Alu.max
                )
                nc.vector.tensor_tensor(dr[:, hc, :], t1[:, hc, :], nbp[:, hc, 1 : W + 1], Alu.max)
                nc.vector.tensor_scalar(
                    erh[:, hc, :], dr[:, hc, :], -1.0, BIGF, Alu.mult, Alu.add
                ).then_inc(dve_s, 1)  # d=2 (c0), d=4 (c1)
            # er_hBT copies for hc0 blocks (land while c1 chain ran)
            vector.wait_ge(pe_s, 3)
            nc.vector.tensor_copy(erTp[:, 0, 1 : 1 + P], blks[4][:])
            vector.wait_ge(pe_s, 4)
            nc.vector.tensor_copy(erTp[:, 1, 1 : 1 + P], blks[5][:])
            # diff = sigmoid(pred) - t (fills the PE-transpose gap)
            vector.wait_ge(act_s, 1)
            nc.vector.tensor_tensor(diff[:], sg[:], tgt[:], Alu.subtract).then_inc(dve_s, 1)  # d=5
            # er_hBT copies for hc1 blocks
            vector.wait_ge(pe_s, 7)
            nc.vector.tensor_copy(erTp[:, 0, 1 + P : 1 + 2 * P], blks[6][:])
            vector.wait_ge(pe_s, 8)
            nc.vector.tensor_copy(erTp[:, 1, 1 + P : 1 + 2 * P], blks[7][:])
            # er3B = min3_v(er_hBT); fv = max(nbT, er3B): 0 on boundary, BIG else
            nc.vector.tensor_tensor(t2[:], erTp[:, :, 0:H], erTp[:, :, 2 : H + 2], Alu.min)
            nc.vector.tensor_tensor(er3[:], t2[:], erTp[:, :, 1 : H + 1], Alu.min)
            vector.wait_ge(act_s, 2)
            nc.vector.tensor_tensor(fvA[:, :, PAD1 : PAD1 + H], er3[:], nbT[:], Alu.max)
            # vertical L1 distance by log-doubling (window 1+2 = 3)
            cur, nxt = fvA, fvB
            for d in (1, 2):
                lo, hi = d, FW - d
                nc.vector.tensor_tensor(
                    tmpd[:, :, lo:hi], cur[:, :, 0 : FW - 2 * d], cur[:, :, 2 * d : FW], Alu.min
                )
                nc.vector.tensor_scalar(
                    tmp2[:, :, lo:hi], tmpd[:, :, lo:hi], float(d), None, Alu.add
                )
                nc.vector.tensor_tensor(
                    nxt[:, :, lo:hi], tmp2[:, :, lo:hi], cur[:, :, lo:hi], Alu.min
                )
                cur, nxt = nxt, cur
            # square the vertical distance, one quarter block at a time so
            # the PE transpose-back pipelines behind each block
            for wb in range(C):
                for hc in range(C):
                    nc.vector.tensor_tensor(
                        g2T[:, wb, hc * P : (hc + 1) * P],
                        cur[:, wb, PAD1 + hc * P : PAD1 + (hc + 1) * P],
                        cur[:, wb, PAD1 + hc * P : PAD1 + (hc + 1) * P],
                        Alu.mult,
                    ).then_inc(dve_s, 1)  # d=6..9
            # g2p copies (all on DVE; ACT copies measured 2x slower)
            for k in range(4):
                wb, hc = divmod(k, C)
                vector.wait_ge(pe_s, 9 + k)
                nc.vector.tensor_copy(g2p[:, hc, KH + wb * P : KH + (wb + 1) * P], blks[k][:])
            # phase 2: d2 = min_{|u|<=KH} g2[j+u] + u^2
            prev = None
            accs = [acc1, acc2, d2]
            for u in range(1, KH + 1):
                nc.vector.tensor_tensor(
                    p2m[:],
                    g2p[:, :, KH - u : KH - u + W],
                    g2p[:, :, KH + u : KH + u + W],
                    Alu.min,
                )
                nc.vector.tensor_scalar(p2h[:], p2m[:], float(u * u), None, Alu.add)
                base = g2p[:, :, KH : KH + W] if prev is None else prev[:]
                nc.vector.tensor_tensor(accs[u - 1][:], p2h[:], base, Alu.min)
                prev = accs[u - 1]
            # ddf = d2 * diff^2 (unblocks the ACT sqrt-accum tail)
            vector.wait_ge(act_s, 3)
            nc.vector.tensor_tensor(ddf[:], d2[:], diff2[:], Alu.mult).then_inc(dve_s, 1)  # d=10
            nc.vector.tensor_reduce(
                out=outb[:, 1:3], in_=d2[:], axis=mybir.AxisListType.X, op=Alu.max
            ).then_inc(dve_s, 1)  # d=11

    return nc


_NC_CACHE = {}


def _get_nc():
    if "nc" not in _NC_CACHE:
        _NC_CACHE["nc"] = _build_nc()
    return _NC_CACHE["nc"]


# ---------- exact numpy fallback (pathological images only) ----------

def _reference_image_np(t, p):
    """Exact replica of the jax reference for one image, in numpy fp32."""
    b = (t > 0.5).astype(np.float32)
    if not (b > 0).any():
        return 0.0
    v = b.copy()
    v[1:] = np.minimum(v[1:], b[:-1])
    v[:-1] = np.minimum(v[:-1], b[1:])
    er = v.copy()
    er[:, 1:] = np.minimum(er[:, 1:], v[:, :-1])
    er[:, :-1] = np.minimum(er[:, :-1], v[:, 1:])
    bound = b - er
    if bound.sum() == 0:
        bound = b
    feat = bound > 0.5
    BIGV = np.float32(1e6)
    c = np.full(W, BIGV, np.float32)
    d_fwd = np.empty((H, W), np.float32)
    for i in range(H):
        c = np.where(feat[i], np.float32(0.0), c + 1)
        d_fwd[i] = c
    c = np.full(W, BIGV, np.float32)
    d_bwd = np.empty((H, W), np.float32)
    for i in range(H - 1, -1, -1):
        c = np.where(feat[i], np.float32(0.0), c + 1)
        d_bwd[i] = c
    g = np.minimum(d_fwd, d_bwd)
    j = np.arange(W, dtype=np.float32)
    d2 = np.empty((H, W), np.float32)
    for i in range(H):
        d2[i] = np.min(g[i][None, :] ** 2 + (j[:, None] - j[None, :]) ** 2, axis=-1)
    dist = np.sqrt(d2)
    m = dist.max()
    if m > 0:
        dist = dist / (m + np.float32(1e-8))
    sgm = 1.0 / (1.0 + np.exp(-p.astype(np.float64)))
    return float(np.mean(dist * np.abs(sgm - t)))


def _bound_empty(t):
    """True if erosion removes every boundary pixel (reference falls back)."""
    b = (t > 0.5).astype(np.float32)
    v = b.copy()
    v[1:] = np.minimum(v[1:], b[:-1])
    v[:-1] = np.minimum(v[:-1], b[1:])
    er = v.copy()
    er[:, 1:] = np.minimum(er[:, 1:], v[:, :-1])
    er[:, :-1] = np.minimum(er[:, :-1], v[:, 1:])
    return (b - er).sum() == 0


# ---------- public entry point ----------

def kernel(pred_logits: np.ndarray, target: np.ndarray) -> np.ndarray:
    global LAST_RESULTS
    import ml_dtypes
    from concourse.bass_utils import run_bass_kernel_spmd

    pred = np.ascontiguousarray(np.asarray(pred_logits, np.float32)[:, 0])
    tgt = np.ascontiguousarray(np.asarray(target, np.float32)[:, 0])
    B = pred.shape[0]
    assert pred.shape == (B, H, W) and tgt.shape == (B, H, W)
    assert B == 8, f"kernel is built for batch 8, got {B}"

    ident = np.eye(P, dtype=ml_dtypes.bfloat16)

    nc = _get_nc()
    in_maps = []
    for i in range(B):
        tp = np.concatenate([tgt[i, :P], tgt[i, P:]], axis=1).astype(ml_dtypes.bfloat16)
        pp = np.concatenate([pred[i, :P], pred[i, P:]], axis=1).astype(ml_dtypes.bfloat16)
        in_maps.append({"tgt": tp, "prd": pp, "idn": ident})
    trace = bool(int(os.environ.get("KERNEL_TRACE", "0")))
    res = run_bass_kernel_spmd(nc, in_maps, core_ids=list(range(B)), trace=trace)
    LAST_RESULTS = res

    total = 0.0
    for i in range(B):
        o = np.asarray(res.results[i]["out"], np.float32)  # [128, 4]
        if not (tgt[i] > 0.5).any():
            continue  # empty mask: reference skips (loss 0)
        m2 = float(o[:, 1:3].max())
        if m2 > float(KH * KH) or _bound_empty(tgt[i]):
            # windowed EDT not provably exact for this image -> exact path
            total += _reference_image_np(tgt[i], pred[i])
            continue
        S = float(o[:, 0].sum(dtype=np.float64))
        m = np.float32(np.sqrt(np.float32(m2)))
        denom = float(m + np.float32(1e-8)) if m > 0 else 1.0
        total += (S / denom) / float(H * W)
    return np.float32(total / max(B, 1))


# revision 11
# speedup vs baseline: 1.0854x; 1.0333x over previous
"""Trainium2 Bass kernel for nn_BoundaryLoss (boundary EDT + weighted L1 loss).

Strategy (pure data parallel, 1 image per NeuronCore, 8 cores):
  Per image on device:
    nbB   = (t <= 0.5) * BIG                  (scaled background mask)
    dr    = max3_h(nbB)                       (horizontal dilation)
    er_hB = BIG - dr                          (scaled horizontal erosion of fg)
    transpose nbB, er_hB (PE)
    er3B  = min3_v(er_hBT)                    (full 3x3 erosion, scaled)
    fv    = max(nbBT, er3B)                   (0 on boundary, BIG elsewhere)
    g     = windowed (+-3) vertical L1 distance via log-doubling
    g2T   = g*g, transpose back (PE)
    d2    = min_{|u|<=3} g2[j+u] + u^2        (windowed exact EDT, K=3)
    diff2 = (sigmoid(pred) - t)^2             (ACT Square)
    outb[:,0]   = sum sqrt(d2*diff2)          (ACT sqrt with accum_out)
    outb[:,1:3] = max d2 per chunk            (DVE reduce)
  Host: verify windowed-exactness bound (m2 <= 9), normalize, batch mean;
  exact numpy fallback for any image failing the bound (never on dense masks).

Engine split: DVE runs the serial mask/EDT chain (bf16, all-SBUF so
tensor_tensor ops hit the 2x mode and tensor_scalar the 4x mode;
scalar_tensor_tensor is avoided -- it runs 1x). ACT handles sigmoid,
4 PSUM copies, Square, and the final fused sqrt+accumulate (one act-table
switch, overlapped with DVE work). PE does the 12 transposes. GpSimd is
kept idle (measured ~6x slower than DVE and it contends for SBUF ports).

All dtypes bf16 except the target (f32, exact 0.5 compare) and the f32
accumulators. Identity ships as a bf16 DRAM tensor (no cast copy).
"""

import os
from contextlib import ExitStack

import numpy as np

H = 256
W = 256
P = 128
C = 2  # partition chunks per image (H = C * P)
KH = 3  # phase-2 horizontal window (exactness proof bound: m2 <= KH*KH)
BIGF = 16384.0  # phase-1 sentinel (bf16-exact; BIGF + small stays BIGF in bf16)
BIG2 = 1.0e9  # phase-2 border pad, > BIGF^2
PAD1 = 4  # phase-1 doubling pads (window 1+2)
FW = H + 2 * PAD1
GW = W + 2 * KH

LAST_RESULTS = None  # BassKernelResults of the most recent device run


def _build_nc():
    import concourse.bass as bass
    import concourse.mybir as mybir

    bf16 = mybir.dt.bfloat16
    f32 = mybir.dt.float32
    Alu = mybir.AluOpType
    Act = mybir.ActivationFunctionType

    nc = bass.Bass(detect_race_conditions=False)
    tgt_d = nc.dram_tensor("tgt", [P, C * W], bf16, kind="ExternalInput")
    prd_d = nc.dram_tensor("prd", [P, C * W], bf16, kind="ExternalInput")
    idn_d = nc.dram_tensor("idn", [P, P], bf16, kind="ExternalInput")
    out_d = nc.dram_tensor("out", [P, 4], f32, kind="ExternalOutput")

    ctx = ExitStack()
    sb = lambda name, shape, dt: ctx.enter_context(nc.sbuf_tensor(name, shape, dt))

    with ctx:
        tgt = sb("tgt_t", [P, C, W], bf16)
        prd = sb("prd_t", [P, C, W], bf16)
        ident = sb("ident", [P, P], bf16)
        nbp = sb("nbp", [P, C, W + 2], bf16)  # padded nbB (pad cols = 0)
        t1 = sb("t1", [P, C, W], bf16)
        dr = sb("dr", [P, C, W], bf16)
        erh = sb("erh", [P, C, W], bf16)
        nbT = sb("nbT", [P, C, H], bf16)
        erTp = sb("erTp", [P, C, H + 2], bf16)  # padded er_hBT (pads = BIGF)
        t2 = sb("t2", [P, C, H], bf16)
        er3 = sb("er3", [P, C, H], bf16)
        fvA = sb("fvA", [P, C, FW], bf16)
        fvB = sb("fvB", [P, C, FW], bf16)
        tmpd = sb("tmpd", [P, C, FW], bf16)
        tmp2 = sb("tmp2", [P, C, FW], bf16)
        g2T = sb("g2T", [P, C, H], bf16)
        g2p = sb("g2p", [P, C, GW], bf16)  # padded g2 (pads = BIG2)
        p2m = sb("p2m", [P, C, W], bf16)
        p2h = sb("p2h", [P, C, W], bf16)
        acc1 = sb("acc1", [P, C, W], bf16)
        acc2 = sb("acc2", [P, C, W], bf16)
        d2 = sb("d2", [P, C, W], bf16)
        sg = sb("sg", [P, C, W], bf16)
        diff = sb("diff", [P, C, W], bf16)
        diff2 = sb("diff2", [P, C, W], bf16)
        ddf = sb("ddf", [P, C, W], bf16)
        junk = sb("junk", [P, C, W], bf16)
        outb = sb("outb", [P, 4], f32)
        warm = sb("warm", [P, 4], f32)
        blks = [ctx.enter_context(nc.psum_tensor(f"blk{i}", [P, P], bf16)) for i in range(8)]

        dma_t = ctx.enter_context(nc.semaphore("dma_t"))
        dma_p = ctx.enter_context(nc.semaphore("dma_p"))
        dma_i = ctx.enter_context(nc.semaphore("dma_i"))
        dma_o = ctx.enter_context(nc.semaphore("dma_o"))
        dve_s = ctx.enter_context(nc.semaphore("dve_s"))
        act_s = ctx.enter_context(nc.semaphore("act_s"))
        pe_s = ctx.enter_context(nc.semaphore("pe_s"))
        w_s = ctx.enter_context(nc.semaphore("w_s"))

        block = ctx.enter_context(nc.Block(no_gpsimd_drain=True))

        # dve_s: 1=nbB(c0) 2=erh(c0) 3=nbB(c1) 4=erh(c1) 5=diff
        #        6-9=g2T quarter blocks 10=ddf 11=m2
        # act_s: 1=sg 2=nbT copied 3=diff2 4=final accum
        # pe_s:  1,2=nbB(hc0) 3,4=er(hc0) 5,6=nbB(hc1) 7,8=er(hc1) 9-12=g2 T
        # psum:  blk0,1=nbB(hc0 wb0/1) blk2,3=nbB(hc1) blk4,5=er(hc0) blk6,7=er(hc1)

        @block.sync
        def _(sync: "bass.BassEngine"):
            # target ships as two chunk DMAs so stage A pipelines per chunk
            sync.dma_start(out=tgt[:, 0, :], in_=tgt_d[:, 0:W]).then_inc(dma_t, 16)
            sync.dma_start(out=tgt[:, 1, :], in_=tgt_d[:, W : 2 * W]).then_inc(dma_t, 16)
            sync.wait_ge(dve_s, 11)
            sync.wait_ge(act_s, 4)
            sync.dma_start(out=out_d[:], in_=outb[:]).then_inc(dma_o, 16)
            sync.wait_ge(dma_o, 16)

        @block.scalar
        def _(scalar: "bass.BassEngine"):
            nc.scalar.dma_start(out=ident[:], in_=idn_d[:]).then_inc(dma_i, 16)
            nc.scalar.dma_start(out=prd[:], in_=prd_d[:]).then_inc(dma_p, 16)
            # warm the sigmoid-set act table while DMAs run (square is in
            # every set, so sigmoid+square need no reload in between)
            scalar.wait_ge(w_s, 1)
            nc.scalar.activation(warm[:, 1:2], warm[:, 0:1], Act.Sigmoid)
            scalar.wait_ge(dma_p, 16)
            nc.scalar.activation(sg[:], prd[:], Act.Sigmoid).then_inc(act_s, 1)  # a=1
            # nbT copies from PSUM banks 0-3 (pe order: 1,2 then 5,6)
            for blk_i, pe_n, wb, hc in ((0, 1, 0, 0), (1, 2, 1, 0), (2, 5, 0, 1), (3, 6, 1, 1)):
                scalar.wait_ge(pe_s, pe_n)
                ins = nc.scalar.copy(nbT[:, wb, hc * P : (hc + 1) * P], blks[blk_i][:])
            ins.then_inc(act_s, 1)  # a=2
            # diff2 = Square(diff) (same act table set)
            scalar.wait_ge(dve_s, 5)
            nc.scalar.activation(diff2[:], diff[:], Act.Square).then_inc(act_s, 1)  # a=3
            # switch table to the sqrt set now (overlaps DVE doubling/phase 2)
            nc.scalar.sqrt(warm[:, 1:2], warm[:, 0:1])
            # final: outb[:,0] = sum sqrt(d2*diff^2) = sum dist*|diff|
            scalar.wait_ge(dve_s, 10)
            nc.scalar.activation(
                junk[:], ddf[:], Act.Sqrt, accum_out=outb[:, 0:1]
            ).then_inc(act_s, 1)  # a=4

        @block.tensor
        def _(tensor: "bass.BassEngine"):
            # per-chunk stage-A transposes, pipelined behind the DVE A-chain
            tensor.wait_ge(dma_i, 16)
            for hc in range(C):
                tensor.wait_ge(dve_s, 1 + 2 * hc)  # nbB(c) ready
                for wb in range(C):
                    nc.tensor.transpose(
                        blks[2 * hc + wb][:], nbp[:, hc, 1 + wb * P : 1 + (wb + 1) * P], ident[:]
                    ).then_inc(pe_s, 1)
                tensor.wait_ge(dve_s, 2 + 2 * hc)  # erh(c) ready
                for wb in range(C):
                    nc.tensor.transpose(
                        blks[4 + 2 * hc + wb][:], erh[:, hc, wb * P : (wb + 1) * P], ident[:]
                    ).then_inc(pe_s, 1)
            # g2 transpose-back: banks 0-3 free once ACT copied nbT (a>=2)
            tensor.wait_ge(act_s, 2)
            for k in range(4):
                wb, hc = divmod(k, C)
                tensor.wait_ge(dve_s, 6 + k)
                nc.tensor.transpose(
                    blks[k][:], g2T[:, wb, hc * P : (hc + 1) * P], ident[:]
                ).then_inc(pe_s, 1)

        @block.vector
        def _(vector: "bass.BassEngine"):
            # data-independent pad memsets (run before input arrives)
            nc.vector.memset(warm[:, 0:1], 1.0).then_inc(w_s, 1)
            nc.vector.memset(nbp[:, :, 0:1], 0.0)
            nc.vector.memset(nbp[:, :, W + 1 : W + 2], 0.0)
            nc.vector.memset(erTp[:, :, 0:1], BIGF)
            nc.vector.memset(erTp[:, :, H + 1 : H + 2], BIGF)
            nc.vector.memset(fvA[:, :, 0:PAD1], BIGF)
            nc.vector.memset(fvA[:, :, PAD1 + H : FW], BIGF)
            nc.vector.memset(fvB[:, :, 0:PAD1], BIGF)
            nc.vector.memset(fvB[:, :, PAD1 + H : FW], BIGF)
            nc.vector.memset(g2p[:, :, 0:KH], BIG2)
            nc.vector.memset(g2p[:, :, KH + W : GW], BIG2)
            nc.vector.memset(outb[:, 3:4], 0.0)

            # per-chunk A-chain: nbB = (t <= 0.5)*BIG, dil_h, er_hB = BIG - dr
            for hc in range(C):
                vector.wait_ge(dma_t, 16 * (hc + 1))
                nc.vector.tensor_scalar(
                    nbp[:, hc, 1 : W + 1], tgt[:, hc, :], 0.5, BIGF, Alu.is_le, Alu.mult
                ).then_inc(dve_s, 1)  # d=1 (c0), d=3 (c1)
                nc.vector.tensor_tensor(
                    t1[:, hc, :], nbp[:, hc, 0:W], nbp[:, hc, 2 : W + 2], Alu.max
                )
                nc.vector.tensor_tensor(dr[:, hc, :], t1[:, hc, :], nbp[:, hc, 1 : W + 1], Alu.max)
                nc.vector.tensor_scalar(
                    erh[:, hc, :], dr[:, hc, :], -1.0, BIGF, Alu.mult, Alu.add
                ).then_inc(dve_s, 1)  # d=2 (c0), d=4 (c1)
            # er_hBT copies for hc0 blocks (land while c1 chain ran)
            vector.wait_ge(pe_s, 3)
            nc.vector.tensor_copy(erTp[:, 0, 1 : 1 + P], blks[4][:])
            vector.wait_ge(pe_s, 4)
            nc.vector.tensor_copy(erTp[:, 1, 1 : 1 + P], blks[5][:])
            # diff = sigmoid(pred) - t (fills the PE-transpose gap)
            vector.wait_ge(act_s, 1)
            nc.vector.tensor_tensor(diff[:], sg[:], tgt[:], Alu.subtract).then_inc(dve_s, 1)  # d=5
            # er_hBT copies for hc1 blocks
            vector.wait_ge(pe_s, 7)
            nc.vector.tensor_copy(erTp[:, 0, 1 + P : 1 + 2 * P], blks[6][:])
            vector.wait_ge(pe_s, 8)
            nc.vector.tensor_copy(erTp[:, 1, 1 + P : 1 + 2 * P], blks[7][:])
            # er3B = min3_v(er_hBT); fv = max(nbT, er3B): 0 on boundary, BIG else
            nc.vector.tensor_tensor(t2[:], erTp[:, :, 0:H], erTp[:, :, 2 : H + 2], Alu.min)
            nc.vector.tensor_tensor(er3[:], t2[:], erTp[:, :, 1 : H + 1], Alu.min)
            vector.wait_ge(act_s, 2)
            nc.vector.tensor_tensor(fvA[:, :, PAD1 : PAD1 + H], er3[:], nbT[:], Alu.max)
            # vertical L1 distance by log-doubling (window 1+2 = 3)
            cur, nxt = fvA, fvB
            for d in (1, 2):
                lo, hi = d, FW - d
                nc.vector.tensor_tensor(
                    tmpd[:, :, lo:hi], cur[:, :, 0 : FW - 2 * d], cur[:, :, 2 * d : FW], Alu.min
                )
                nc.vector.tensor_scalar(
                    tmp2[:, :, lo:hi], tmpd[:, :, lo:hi], float(d), None, Alu.add
                )
                nc.vector.tensor_tensor(
                    nxt[:, :, lo:hi], tmp2[:, :, lo:hi], cur[:, :, lo:hi], Alu.min
                )
                cur, nxt = nxt, cur
            # square the vertical distance, one quarter block at a time so
            # the PE transpose-back pipelines behind each block
            for wb in range(C):
                for hc in range(C):
                    nc.vector.tensor_tensor(
                        g2T[:, wb, hc * P : (hc + 1) * P],
                        cur[:, wb, PAD1 + hc * P : PAD1 + (hc + 1) * P],
                        cur[:, wb, PAD1 + hc * P : PAD1 + (hc + 1) * P],
                        Alu.mult,
                    ).then_inc(dve_s, 1)  # d=6..9
            # g2p copies (all on DVE; ACT copies measured 2x slower)
            for k in range(4):
                wb, hc = divmod(k, C)
                vector.wait_ge(pe_s, 9 + k)
                nc.vector.tensor_copy(g2p[:, hc, KH + wb * P : KH + (wb + 1) * P], blks[k][:])
            # phase 2: d2 = min_{|u|<=KH} g2[j+u] + u^2
            prev = None
            accs = [acc1, acc2, d2]
            for u in range(1, KH + 1):
                nc.vector.tensor_tensor(
                    p2m[:],
                    g2p[:, :, KH - u : KH - u + W],
                    g2p[:, :, KH + u : KH + u + W],
                    Alu.min,
                )
                nc.vector.tensor_scalar(p2h[:], p2m[:], float(u * u), None, Alu.add)
                base = g2p[:, :, KH : KH + W] if prev is None else prev[:]
                nc.vector.tensor_tensor(accs[u - 1][:], p2h[:], base, Alu.min)
                prev = accs[u - 1]
            # ddf = d2 * diff^2 (unblocks the ACT sqrt-accum tail)
            vector.wait_ge(act_s, 3)
            nc.vector.tensor_tensor(ddf[:], d2[:], diff2[:], Alu.mult).then_inc(dve_s, 1)  # d=10
            nc.vector.tensor_reduce(
                out=outb[:, 1:3], in_=d2[:], axis=mybir.AxisListType.X, op=Alu.max
            ).then_inc(dve_s, 1)  # d=11

    return nc


_NC_CACHE = {}


def _get_nc():
    if "nc" not in _NC_CACHE:
        _NC_CACHE["nc"] = _build_nc()
    return _NC_CACHE["nc"]


# ---------- exact numpy fallback (pathological images only) ----------

def _reference_image_np(t, p):
    """Exact replica of the jax reference for one image, in numpy fp32."""
    b = (t > 0.5).astype(np.float32)
    if not (b > 0).any():
        return 0.0
    v = b.copy()
    v[1:] = np.minimum(v[1:], b[:-1])
    v[:-1] = np.minimum(v[:-1], b[1:])
    er = v.copy()
    er[:, 1:] = np.minimum(er[:, 1:], v[:, :-1])
    er[:, :-1] = np.minimum(er[:, :-1], v[:, 1:])
    bound = b - er
    if bound.sum() == 0:
        bound = b
    feat = bound > 0.5
    BIGV = np.float32(1e6)
    c = np.full(W, BIGV, np.float32)
    d_fwd = np.empty((H, W), np.float32)
    for i in range(H):
        c = np.where(feat[i], np.float32(0.0), c + 1)
        d_fwd[i] = c
    c = np.full(W, BIGV, np.float32)
    d_bwd = np.empty((H, W), np.float32)
    for i in range(H - 1, -1, -1):
        c = np.where(feat[i], np.float32(0.0), c + 1)
        d_bwd[i] = c
    g = np.minimum(d_fwd, d_bwd)
    j = np.arange(W, dtype=np.float32)
    d2 = np.empty((H, W), np.float32)
    for i in range(H):
        d2[i] = np.min(g[i][None, :] ** 2 + (j[:, None] - j[None, :]) ** 2, axis=-1)
    dist = np.sqrt(d2)
    m = dist.max()
    if m > 0:
        dist = dist / (m + np.float32(1e-8))
    sgm = 1.0 / (1.0 + np.exp(-p.astype(np.float64)))
    return float(np.mean(dist * np.abs(sgm - t)))


def _bound_empty(t):
    """True if erosion removes every boundary pixel (reference falls back)."""
    b = (t > 0.5).astype(np.float32)
    v = b.copy()
    v[1:] = np.minimum(v[1:], b[:-1])
    v[:-1] = np.minimum(v[:-1], b[1:])
    er = v.copy()
    er[:, 1:] = np.minimum(er[:, 1:], v[:, :-1])
    er[:, :-1] = np.minimum(er[:, :-1], v[:, 1:])
    return (b - er).sum() == 0


# ---------- public entry point ----------

def kernel(pred_logits: np.ndarray, target: np.ndarray) -> np.ndarray:
    global LAST_RESULTS
    import ml_dtypes
    from concourse.bass_utils import run_bass_kernel_spmd

    pred = np.ascontiguousarray(np.asarray(pred_logits, np.float32)[:, 0])
    tgt = np.ascontiguousarray(np.asarray(target, np.float32)[:, 0])
    B = pred.shape[0]
    assert pred.shape == (B, H, W) and tgt.shape == (B, H, W)
    assert B == 8, f"kernel is built for batch 8, got {B}"

    ident = np.eye(P, dtype=ml_dtypes.bfloat16)

    nc = _get_nc()
    in_maps = []
    for i in range(B):
        tp = np.concatenate([tgt[i, :P], tgt[i, P:]], axis=1).astype(ml_dtypes.bfloat16)
        pp = np.concatenate([pred[i, :P], pred[i, P:]], axis=1).astype(ml_dtypes.bfloat16)
        in_maps.append({"tgt": tp, "prd": pp, "idn": ident})
    trace = bool(int(os.environ.get("KERNEL_TRACE", "0")))
    res = run_bass_kernel_spmd(nc, in_maps, core_ids=list(range(B)), trace=trace)
    LAST_RESULTS = res

    total = 0.0
    for i in range(B):
        o = np.asarray(res.results[i]["out"], np.float32)  # [128, 4]
        if not (tgt[i] > 0.5).any():
            continue  # empty mask: reference skips (loss 0)
        m2 = float(o[:, 1:3].max())
        if m2 > float(KH * KH) or _bound_empty(tgt[i]):
            # windowed EDT not provably exact for this image -> exact path
            total += _reference_image_np(tgt[i], pred[i])
            continue
        S = float(o[:, 0].sum(dtype=np.float64))
        m = np.float32(np.sqrt(np.float32(m2)))
        denom = float(m + np.float32(1e-8)) if m > 0 else 1.0
        total += (S / denom) / float(H * W)
    return np.float32(total / max(B, 1))
